# revision 8
# baseline (speedup 1.0000x reference)
"""Chamfer loss kernel for TRN2 (8 NeuronCores, data-parallel over batch).

Reference computation (per batch b):
  t = l2_normalize(tokens[b])      # (K=1024, D=128)
  i = l2_normalize(interests[b])   # (M=64,  D=128)
  dist[k,m] = sqrt(2 - 2*dot(t_k, i_m))   (since ||t||=||i||=1)
  loss = mean_bm(min_k dist) + 0.3 * mean_bk(min_m dist)

min dist <=> max dot: reduce max over normalized dots, apply sqrt(2-2x)
only to tiny reduced tensors.

Per-core structure (64 batches each):
  phase 0 (pipelined in 4 chunks): normalize all interests, transpose to
    iT_all [128d, (b, 64m)] bf16.
  per batch:
    DMA   tokens[b] -> t_all [128,(8,128)] fp32
    PE    8 transposes (f32r, raw) -> ptT psum
    ACT   evacuate + downcast -> tT bf16 sbuf
    DVE   t2T = tT*tT (bf16 2x mode)
    PE    8 mini-matmuls (ones rhs, ap=1) -> per-token sumsq in psum
    ACT   sqrt -> tnrm; DVE reciprocal -> invt [128,8]
    PE    8 dots matmuls (bf16): pdots[k,(n m)] = tT.T @ iT_b
    DVE+POOL  dn = pdots * invt (split halves, evacuates psum, bf16)
    DVE   st_t = max over m (free reduce)
    POOL  partition-max over k-partitions -> nmax (replicated)
    DVE   max-tree over n (bf16 2x) -> st_i
    every 4 batches: ACT sqrt(2-2x) on staged maxes, DVE accumulate
Host combines the 8 per-core partial sums.
"""

import numpy as np
from contextlib import ExitStack

import concourse.bass as bass
import concourse.bass_isa as bass_isa
import concourse.mybir as mybir
import concourse.tile as tile
from concourse import bacc
from concourse.bass_utils import run_bass_kernel_spmd

N_CORES = 8
B, K, M, D = 512, 1024, 64, 128
B_LOC = B // N_CORES          # 64 batches per core
KT = K // 128                 # 8 token tiles of [128, D] per batch
ALPHA_T_TO_I = 0.3
STG = 4                       # sqrt-staging factor (batches per sqrt op)
LAG = 3                       # tail lags front by LAG batches

F32 = mybir.dt.float32
F32R = mybir.dt.float32r
BF16 = mybir.dt.bfloat16
AX = mybir.AxisListType
OP = mybir.AluOpType
ACT = mybir.ActivationFunctionType
RED = bass_isa.ReduceOp


def build(b_loc=B_LOC):
    assert b_loc % STG == 0 and b_loc % 16 == 0
    nc = bacc.Bacc(
        "TRN2",
        target_bir_lowering=False,
        debug=False,
        num_devices=N_CORES,
    )
    tokens = nc.dram_tensor("tokens", [b_loc, K, D], F32, kind="ExternalInput").ap()
    interests = nc.dram_tensor(
        "interests", [b_loc, M, D], F32, kind="ExternalInput"
    ).ap()
    out = nc.dram_tensor("out", [1, 2], F32, kind="ExternalOutput").ap()

    NG = b_loc * M // 128     # interest row-groups of 128 (= b_loc/2)
    NCHUNK = 4                # phase-0 chunks
    CG = NG // NCHUNK         # groups per chunk (8)

    with ExitStack() as ctx:
        tc = ctx.enter_context(tile.TileContext(nc))
        singles = ctx.enter_context(tc.tile_pool(name="singles", bufs=1))
        tok_pool = ctx.enter_context(tc.tile_pool(name="tok", bufs=5))
        tT_pool = ctx.enter_context(tc.tile_pool(name="tT", bufs=3))
        t2_pool = ctx.enter_context(tc.tile_pool(name="t2", bufs=2))
        dn_pool = ctx.enter_context(tc.tile_pool(name="dn", bufs=6))
        nm_pool = ctx.enter_context(tc.tile_pool(name="nm", bufs=3))
        tr_pool = ctx.enter_context(tc.tile_pool(name="tr", bufs=3))
        small = ctx.enter_context(tc.tile_pool(name="small", bufs=8))
        stage = ctx.enter_context(tc.tile_pool(name="stage", bufs=3))
        p_tT = ctx.enter_context(tc.tile_pool(name="p_tT", bufs=2, space="PSUM"))
        p_dots = ctx.enter_context(tc.tile_pool(name="p_dots", bufs=2, space="PSUM"))
        p_sum = ctx.enter_context(tc.tile_pool(name="p_sum", bufs=1, space="PSUM"))

        # identities for PE transposes (f32r for tokens, bf16 for interests)
        id_f32 = singles.tile([128, 128], F32)
        nc.gpsimd.memset(id_f32, 0.0)
        nc.gpsimd.affine_select(
            out=id_f32, in_=id_f32, compare_op=OP.not_equal, fill=1.0,
            base=0, pattern=[[-1, 128]], channel_multiplier=1,
        )
        id_bf = singles.tile([128, 128], BF16)
        nc.gpsimd.memset(id_bf, 0.0)
        nc.gpsimd.affine_select(
            out=id_bf, in_=id_bf, compare_op=OP.not_equal, fill=1.0,
            base=0, pattern=[[-1, 128]], channel_multiplier=1,
        )
        ones_bf = singles.tile([128, 1], BF16)
        nc.vector.memset(ones_bf, 1.0)
        ones_f = singles.tile([128, 1], F32)
        nc.vector.memset(ones_f, 1.0)
        two = singles.tile([128, 1], F32)
        nc.vector.memset(two, 2.0)
        acc_t = singles.tile([128, STG * KT], F32)
        acc_i = singles.tile([128, STG * M], F32)
        nc.vector.memset(acc_t, 0.0)
        nc.vector.memset(acc_i, 0.0)

        # ---------- phase 0: all interests -> normalized iT_all (bf16) ----------
        i_flat = interests.rearrange("b m d -> (b m) d").rearrange(
            "(g p) d -> p g d", p=128
        )  # [128, NG, 128]
        i_all = singles.tile([128, NG, D], F32)
        nc.sync.dma_start(out=i_all, in_=i_flat)
        iT_all = singles.tile([128, b_loc, M], BF16)   # [d, (b, m)]

        for c in range(NCHUNK):
            sl = i_all[:, c * CG:(c + 1) * CG, :]          # [128, 8, 128]
            i2 = t2_pool.tile([128, CG, D], BF16, tag="i2")
            nc.scalar.square(i2, sl)
            isum = small.tile([128, CG], F32, tag="isum")
            nc.vector.tensor_reduce(isum, i2, axis=AX.X, op=OP.add)
            inrm = small.tile([128, CG], F32, tag="inrm")
            nc.scalar.sqrt(inrm, isum)
            invi = small.tile([128, CG], F32, tag="invi")
            nc.vector.reciprocal(invi, inrm)
            i_nb = tT_pool.tile([128, CG, D], BF16, tag="i_nb")
            nc.gpsimd.tensor_mul(i_nb, sl, invi.broadcast_to([128, CG, D]))
            ptiT = p_sum.tile([128, CG, 128], BF16, tag="ptiT")
            for j in range(CG):
                nc.tensor.transpose(ptiT[:, j, :], i_nb[:, j, :], id_bf)
            dst = iT_all[:, c * 2 * CG:(c + 1) * 2 * CG, :].rearrange(
                "p a b -> p (a b)"
            )
            src = ptiT.rearrange("p a b -> p (a b)")
            if c % 2 == 0:
                nc.vector.tensor_copy(dst, src)
            else:
                nc.scalar.copy(dst, src)

        # ---------- main loop (software-pipelined) ----------
        dn_of = {}
        st_of = {}

        def front(b):
            t_all = tok_pool.tile([128, KT, D], F32)
            nc.sync.dma_start(
                out=t_all, in_=tokens[b].rearrange("(n p) d -> p n d", p=128)
            )
            # transposes of raw token tiles (fp32)
            ptT = p_tT.tile([128, KT, 128], F32)
            for n in range(KT):
                nc.tensor.transpose(ptT[:, n, :], t_all[:, n, :], id_f32)
            # evacuate + downcast on ACT
            tT = tT_pool.tile([128, KT, 128], BF16, tag="tT")
            nc.scalar.copy(
                tT.rearrange("p a b -> p (a b)"),
                ptT.rearrange("p a b -> p (a b)"),
            )
            # squares (DVE bf16 2x) then per-token sumsq via mini-matmuls
            t2T = t2_pool.tile([128, KT, 128], BF16, tag="t2T")
            nc.vector.tensor_mul(t2T, tT, tT)
            ptsum = p_sum.tile([128, KT], F32)
            for n in range(KT):
                nc.tensor.matmul(
                    ptsum[:, n:n + 1], lhsT=t2T[:, n, :], rhs=ones_bf,
                    start=True, stop=True,
                )
            tnrm = small.tile([128, KT], F32, tag="tnrm")
            nc.scalar.sqrt(tnrm, ptsum)
            invt = small.tile([128, KT], F32, tag="invt")
            nc.vector.reciprocal(invt, tnrm)
            # dots (raw bf16 tokens x normalized interests)
            pdots = p_dots.tile([128, KT, M], F32, tag="pd")
            iT = iT_all[:, b, :]
            for n in range(KT):
                nc.tensor.matmul(
                    pdots[:, n, :], lhsT=tT[:, n, :], rhs=iT,
                    start=True, stop=True,
                )
            # normalize + evacuate (DVE; Pool cannot read PSUM)
            dn = dn_pool.tile([128, KT, M], BF16)
            nc.vector.tensor_mul(
                dn, pdots, invt.broadcast_to([128, KT, M]),
            )
            dn_of[b] = dn

        def tail(bb):
            s2 = bb % STG
            g = bb // STG
            if s2 == 0:
                st_t_new = stage.tile([128, STG, KT], F32, tag="st_t")
                st_i_new = stage.tile([128, STG, M], BF16, tag="st_i")
                st_of[g] = (st_t_new, st_i_new)
            st_t, st_i = st_of[g]
            dn = dn_of.pop(bb)
            # per-token max over m
            nc.vector.tensor_reduce(st_t[:, s2, :], dn, axis=AX.X, op=OP.max)
            # per-interest: partition-max then max-tree over n
            nmax = nm_pool.tile([128, KT, M], BF16)
            nc.gpsimd.partition_all_reduce(
                nmax.rearrange("p a b -> p (a b)"),
                dn.rearrange("p a b -> p (a b)"),
                channels=128, reduce_op=RED.max,
            )
            m1 = tr_pool.tile([128, KT // 2, M], BF16, tag="m1")
            nc.vector.tensor_max(m1, nmax[:, 0:4, :], nmax[:, 4:8, :])
            m2 = tr_pool.tile([128, KT // 4, M], BF16, tag="m2")
            nc.vector.tensor_max(m2, m1[:, 0:2, :], m1[:, 2:4, :])
            nc.vector.tensor_max(st_i[:, s2, :], m2[:, 0, :], m2[:, 1, :])
            if s2 == STG - 1:
                del st_of[g]
                dts = stage.tile([128, STG * KT], F32, tag="dts")
                nc.scalar.activation(
                    dts, st_t.rearrange("p a b -> p (a b)"),
                    ACT.Sqrt, bias=two[:], scale=-2.0,
                )
                nc.gpsimd.tensor_add(acc_t, acc_t, dts)
                dis = stage.tile([128, STG * M], F32, tag="dis")
                nc.scalar.activation(
                    dis, st_i.rearrange("p a b -> p (a b)"),
                    ACT.Sqrt, bias=two[:], scale=-2.0,
                )
                nc.gpsimd.tensor_add(acc_i, acc_i, dis)

        for vb in range(b_loc + LAG):
            if vb >= LAG:
                tail(vb - LAG)
            if vb < b_loc:
                front(vb)

        # ---------- final reductions ----------
        red_t = singles.tile([128, 1], F32)
        nc.vector.tensor_reduce(red_t, acc_t, axis=AX.X, op=OP.add)
        pfin3 = p_dots.tile([128, KT, M], F32, tag="pd")
        pfin = pfin3.rearrange("p a b -> p (a b)")
        nc.tensor.matmul(pfin[:1, :1], lhsT=ones_f, rhs=red_t, start=True, stop=True)
        red_i = singles.tile([128, 1], F32)
        nc.vector.tensor_reduce(red_i, acc_i, axis=AX.X, op=OP.add)
        out_sb = small.tile([1, 2], F32, tag="out_sb")
        nc.scalar.copy(out_sb[:, 0:1], pfin[:1, :1])
        nc.scalar.copy(out_sb[:, 1:2], red_i[0:1, :])
        nc.sync.dma_start(out=out, in_=out_sb)

    nc.compile()
    return nc


_NC_CACHE = None


def _get_nc():
    global _NC_CACHE
    if _NC_CACHE is None:
        _NC_CACHE = build()
    return _NC_CACHE


def kernel(tokens: np.ndarray, interests: np.ndarray, _trace=False) -> np.ndarray:
    tokens = np.ascontiguousarray(tokens, dtype=np.float32)
    interests = np.ascontiguousarray(interests, dtype=np.float32)
    assert tokens.shape == (B, K, D) and interests.shape == (B, M, D)

    nc = _get_nc()
    in_maps = [
        {
            "tokens": tokens[c * B_LOC:(c + 1) * B_LOC],
            "interests": interests[c * B_LOC:(c + 1) * B_LOC],
        }
        for c in range(N_CORES)
    ]
    res = run_bass_kernel_spmd(
        nc, in_maps, core_ids=list(range(N_CORES)), trace=_trace
    )
    sum_t = 0.0  # sum over all (b, k) of min_m dist
    sum_i = 0.0  # sum over all (b, m) of min_k dist
    for r in res.results:
        sum_t += float(r["out"][0, 0])
        sum_i += float(r["out"][0, 1])
    loss = sum_i / (B * M) + ALPHA_T_TO_I * sum_t / (B * K)
    kernel.last_results = res
    return np.array(loss, dtype=np.float32)


# revision 9
# speedup vs baseline: 1.0218x; 1.0218x over previous
"""Chamfer loss kernel for TRN2 (8 NeuronCores, data-parallel over batch).

Reference computation (per batch b):
  t = l2_normalize(tokens[b])      # (K=1024, D=128)
  i = l2_normalize(interests[b])   # (M=64,  D=128)
  dist[k,m] = sqrt(2 - 2*dot(t_k, i_m))   (since ||t||=||i||=1)
  loss = mean_bm(min_k dist) + 0.3 * mean_bk(min_m dist)

min dist <=> max dot: reduce max over normalized dots, apply sqrt(2-2x)
only to tiny reduced tensors.

Per-core structure (64 batches each):
  phase 0 (pipelined in 4 chunks): normalize all interests, transpose to
    iT_all [128d, (b, 64m)] bf16.
  per batch:
    DMA   tokens[b] -> t_all [128,(8,128)] fp32
    PE    8 transposes (f32r, raw) -> ptT psum
    ACT   evacuate + downcast -> tT bf16 sbuf
    DVE   t2T = tT*tT (bf16 2x mode)
    PE    8 mini-matmuls (ones rhs, ap=1) -> per-token sumsq in psum
    ACT   sqrt -> tnrm; DVE reciprocal -> invt [128,8]
    PE    8 dots matmuls (bf16): pdots[k,(n m)] = tT.T @ iT_b
    DVE+POOL  dn = pdots * invt (split halves, evacuates psum, bf16)
    DVE   st_t = max over m (free reduce)
    POOL  partition-max over k-partitions -> nmax (replicated)
    DVE   max-tree over n (bf16 2x) -> st_i
    every 4 batches: ACT sqrt(2-2x) on staged maxes, DVE accumulate
Host combines the 8 per-core partial sums.
"""

import numpy as np
from contextlib import ExitStack

import concourse.bass as bass
import concourse.bass_isa as bass_isa
import concourse.mybir as mybir
import concourse.tile as tile
from concourse import bacc
from concourse.bass_utils import run_bass_kernel_spmd

N_CORES = 8
B, K, M, D = 512, 1024, 64, 128
B_LOC = B // N_CORES          # 64 batches per core
KT = K // 128                 # 8 token tiles of [128, D] per batch
ALPHA_T_TO_I = 0.3
STG = 4                       # sqrt-staging factor (batches per sqrt op)
LAG = 3                       # tail lags front by LAG batches

F32 = mybir.dt.float32
F32R = mybir.dt.float32r
BF16 = mybir.dt.bfloat16
AX = mybir.AxisListType
OP = mybir.AluOpType
ACT = mybir.ActivationFunctionType
RED = bass_isa.ReduceOp


def build(b_loc=B_LOC):
    assert b_loc % STG == 0 and b_loc % 16 == 0
    nc = bacc.Bacc(
        "TRN2",
        target_bir_lowering=False,
        debug=False,
        num_devices=N_CORES,
    )
    tokens = nc.dram_tensor("tokens", [b_loc, K, D], F32, kind="ExternalInput").ap()
    interests = nc.dram_tensor(
        "interests", [b_loc, M, D], F32, kind="ExternalInput"
    ).ap()
    out = nc.dram_tensor("out", [1, 2], F32, kind="ExternalOutput").ap()

    NG = b_loc * M // 128     # interest row-groups of 128 (= b_loc/2)
    NCHUNK = 4                # phase-0 chunks
    CG = NG // NCHUNK         # groups per chunk (8)

    with ExitStack() as ctx:
        tc = ctx.enter_context(tile.TileContext(nc))
        singles = ctx.enter_context(tc.tile_pool(name="singles", bufs=1))
        tok_pool = ctx.enter_context(tc.tile_pool(name="tok", bufs=5))
        tT_pool = ctx.enter_context(tc.tile_pool(name="tT", bufs=3))
        t2_pool = ctx.enter_context(tc.tile_pool(name="t2", bufs=2))
        dn_pool = ctx.enter_context(tc.tile_pool(name="dn", bufs=6))
        nm_pool = ctx.enter_context(tc.tile_pool(name="nm", bufs=3))
        tr_pool = ctx.enter_context(tc.tile_pool(name="tr", bufs=3))
        small = ctx.enter_context(tc.tile_pool(name="small", bufs=8))
        stage = ctx.enter_context(tc.tile_pool(name="stage", bufs=3))
        p_tT = ctx.enter_context(tc.tile_pool(name="p_tT", bufs=2, space="PSUM"))
        p_dots = ctx.enter_context(tc.tile_pool(name="p_dots", bufs=2, space="PSUM"))
        p_sum = ctx.enter_context(tc.tile_pool(name="p_sum", bufs=1, space="PSUM"))

        # identities for PE transposes (f32r for tokens, bf16 for interests)
        id_f32 = singles.tile([128, 128], F32)
        nc.gpsimd.memset(id_f32, 0.0)
        nc.gpsimd.affine_select(
            out=id_f32, in_=id_f32, compare_op=OP.not_equal, fill=1.0,
            base=0, pattern=[[-1, 128]], channel_multiplier=1,
        )
        id_bf = singles.tile([128, 128], BF16)
        nc.gpsimd.memset(id_bf, 0.0)
        nc.gpsimd.affine_select(
            out=id_bf, in_=id_bf, compare_op=OP.not_equal, fill=1.0,
            base=0, pattern=[[-1, 128]], channel_multiplier=1,
        )
        ones_bf = singles.tile([128, 1], BF16)
        nc.vector.memset(ones_bf, 1.0)
        ones_f = singles.tile([128, 1], F32)
        nc.vector.memset(ones_f, 1.0)
        two = singles.tile([128, 1], F32)
        nc.vector.memset(two, 2.0)
        acc_t = singles.tile([128, STG * KT], F32)
        acc_i = singles.tile([128, STG * M], F32)
        nc.vector.memset(acc_t, 0.0)
        nc.vector.memset(acc_i, 0.0)

        # ---------- phase 0: all interests -> normalized iT_all (bf16) ----------
        i_flat = interests.rearrange("b m d -> (b m) d").rearrange(
            "(g p) d -> p g d", p=128
        )  # [128, NG, 128]
        i_all = singles.tile([128, NG, D], F32)
        nc.sync.dma_start(out=i_all, in_=i_flat)
        iT_all = singles.tile([128, b_loc, M], BF16)   # [d, (b, m)]

        for c in range(NCHUNK):
            sl = i_all[:, c * CG:(c + 1) * CG, :]          # [128, 8, 128]
            i2 = t2_pool.tile([128, CG, D], BF16, tag="i2")
            nc.scalar.square(i2, sl)
            isum = small.tile([128, CG], F32, tag="isum")
            nc.vector.tensor_reduce(isum, i2, axis=AX.X, op=OP.add)
            inrm = small.tile([128, CG], F32, tag="inrm")
            nc.scalar.sqrt(inrm, isum)
            invi = small.tile([128, CG], F32, tag="invi")
            nc.vector.reciprocal(invi, inrm)
            i_nb = tT_pool.tile([128, CG, D], BF16, tag="i_nb")
            nc.gpsimd.tensor_mul(i_nb, sl, invi.broadcast_to([128, CG, D]))
            ptiT = p_sum.tile([128, CG, 128], BF16, tag="ptiT")
            for j in range(CG):
                nc.tensor.transpose(ptiT[:, j, :], i_nb[:, j, :], id_bf)
            dst = iT_all[:, c * 2 * CG:(c + 1) * 2 * CG, :].rearrange(
                "p a b -> p (a b)"
            )
            src = ptiT.rearrange("p a b -> p (a b)")
            if c % 2 == 0:
                nc.vector.tensor_copy(dst, src)
            else:
                nc.scalar.copy(dst, src)

        # ---------- main loop (software-pipelined) ----------
        dn_of = {}
        st_of = {}

        def front(b):
            t_all = tok_pool.tile([128, KT, D], F32)
            nc.sync.dma_start(
                out=t_all, in_=tokens[b].rearrange("(n p) d -> p n d", p=128)
            )
            # transposes of raw token tiles (fp32)
            ptT = p_tT.tile([128, KT, 128], F32)
            for n in range(KT):
                nc.tensor.transpose(ptT[:, n, :], t_all[:, n, :], id_f32)
            # evacuate + downcast on ACT
            tT = tT_pool.tile([128, KT, 128], BF16, tag="tT")
            nc.scalar.copy(
                tT.rearrange("p a b -> p (a b)"),
                ptT.rearrange("p a b -> p (a b)"),
            )
            # squares (DVE bf16 2x) then per-token sumsq via mini-matmuls
            t2T = t2_pool.tile([128, KT, 128], BF16, tag="t2T")
            nc.vector.tensor_mul(t2T[:, 0:6, :], tT[:, 0:6, :], tT[:, 0:6, :])
            nc.gpsimd.tensor_mul(t2T[:, 6:8, :], tT[:, 6:8, :], tT[:, 6:8, :])
            ptsum = p_sum.tile([128, KT], F32)
            for n in range(KT):
                nc.tensor.matmul(
                    ptsum[:, n:n + 1], lhsT=t2T[:, n, :], rhs=ones_bf,
                    start=True, stop=True,
                )
            tnrm = small.tile([128, KT], F32, tag="tnrm")
            nc.scalar.sqrt(tnrm, ptsum)
            invt = small.tile([128, KT], F32, tag="invt")
            nc.vector.reciprocal(invt, tnrm)
            # dots (raw bf16 tokens x normalized interests)
            pdots = p_dots.tile([128, KT, M], F32, tag="pd")
            iT = iT_all[:, b, :]
            for n in range(KT):
                nc.tensor.matmul(
                    pdots[:, n, :], lhsT=tT[:, n, :], rhs=iT,
                    start=True, stop=True,
                )
            # normalize + evacuate (DVE; Pool cannot read PSUM)
            dn = dn_pool.tile([128, KT, M], BF16)
            nc.vector.tensor_mul(
                dn, pdots, invt.broadcast_to([128, KT, M]),
            )
            dn_of[b] = dn

        def tail(bb):
            s2 = bb % STG
            g = bb // STG
            if s2 == 0:
                st_t_new = stage.tile([128, STG, KT], F32, tag="st_t")
                st_i_new = stage.tile([128, STG, M], BF16, tag="st_i")
                st_of[g] = (st_t_new, st_i_new)
            st_t, st_i = st_of[g]
            dn = dn_of.pop(bb)
            # per-token max over m
            nc.vector.tensor_reduce(st_t[:, s2, :], dn, axis=AX.X, op=OP.max)
            # per-interest: max-tree level 1, partition-max on half, rest of tree
            m1 = tr_pool.tile([128, KT // 2, M], BF16, tag="m1")
            nc.vector.tensor_max(m1, dn[:, 0:4, :], dn[:, 4:8, :])
            nmax = nm_pool.tile([128, KT // 2, M], BF16)
            nc.gpsimd.partition_all_reduce(
                nmax.rearrange("p a b -> p (a b)"),
                m1.rearrange("p a b -> p (a b)"),
                channels=128, reduce_op=RED.max,
            )
            m2 = tr_pool.tile([128, KT // 4, M], BF16, tag="m2")
            nc.vector.tensor_max(m2, nmax[:, 0:2, :], nmax[:, 2:4, :])
            nc.vector.tensor_max(st_i[:, s2, :], m2[:, 0, :], m2[:, 1, :])
            if s2 == STG - 1:
                del st_of[g]
                dts = stage.tile([128, STG * KT], F32, tag="dts")
                nc.scalar.activation(
                    dts, st_t.rearrange("p a b -> p (a b)"),
                    ACT.Sqrt, bias=two[:], scale=-2.0,
                )
                nc.gpsimd.tensor_add(acc_t, acc_t, dts)
                dis = stage.tile([128, STG * M], F32, tag="dis")
                nc.scalar.activation(
                    dis, st_i.rearrange("p a b -> p (a b)"),
                    ACT.Sqrt, bias=two[:], scale=-2.0,
                )
                nc.gpsimd.tensor_add(acc_i, acc_i, dis)

        for vb in range(b_loc + LAG):
            if vb >= LAG:
                tail(vb - LAG)
            if vb < b_loc:
                front(vb)

        # ---------- final reductions ----------
        red_t = singles.tile([128, 1], F32)
        nc.vector.tensor_reduce(red_t, acc_t, axis=AX.X, op=OP.add)
        pfin3 = p_dots.tile([128, KT, M], F32, tag="pd")
        pfin = pfin3.rearrange("p a b -> p (a b)")
        nc.tensor.matmul(pfin[:1, :1], lhsT=ones_f, rhs=red_t, start=True, stop=True)
        red_i = singles.tile([128, 1], F32)
        nc.vector.tensor_reduce(red_i, acc_i, axis=AX.X, op=OP.add)
        out_sb = small.tile([1, 2], F32, tag="out_sb")
        nc.scalar.copy(out_sb[:, 0:1], pfin[:1, :1])
        nc.scalar.copy(out_sb[:, 1:2], red_i[0:1, :])
        nc.sync.dma_start(out=out, in_=out_sb)

    nc.compile()
    return nc


_NC_CACHE = None


def _get_nc():
    global _NC_CACHE
    if _NC_CACHE is None:
        _NC_CACHE = build()
    return _NC_CACHE


def kernel(tokens: np.ndarray, interests: np.ndarray, _trace=False) -> np.ndarray:
    tokens = np.ascontiguousarray(tokens, dtype=np.float32)
    interests = np.ascontiguousarray(interests, dtype=np.float32)
    assert tokens.shape == (B, K, D) and interests.shape == (B, M, D)

    nc = _get_nc()
    in_maps = [
        {
            "tokens": tokens[c * B_LOC:(c + 1) * B_LOC],
            "interests": interests[c * B_LOC:(c + 1) * B_LOC],
        }
        for c in range(N_CORES)
    ]
    res = run_bass_kernel_spmd(
        nc, in_maps, core_ids=list(range(N_CORES)), trace=_trace
    )
    sum_t = 0.0  # sum over all (b, k) of min_m dist
    sum_i = 0.0  # sum over all (b, m) of min_k dist
    for r in res.results:
        sum_t += float(r["out"][0, 0])
        sum_i += float(r["out"][0, 1])
    loss = sum_i / (B * M) + ALPHA_T_TO_I * sum_t / (B * K)
    kernel.last_results = res
    return np.array(loss, dtype=np.float32)


# revision 10
# speedup vs baseline: 1.1447x; 1.1203x over previous
"""Chamfer loss kernel for TRN2 (8 NeuronCores, data-parallel over batch).

Reference computation (per batch b):
  t = l2_normalize(tokens[b])      # (K=1024, D=128)
  i = l2_normalize(interests[b])   # (M=64,  D=128)
  dist[k,m] = sqrt(2 - 2*dot(t_k, i_m))   (since ||t||=||i||=1)
  loss = mean_bm(min_k dist) + 0.3 * mean_bk(min_m dist)

min dist <=> max dot: reduce max over normalized dots, apply sqrt(2-2x)
only to tiny reduced tensors.

Per-core structure (64 batches each):
  phase 0 (pipelined in 4 chunks): normalize all interests, transpose to
    iT_all [128d, (b, 64m)] bf16.
  per batch:
    DMA   tokens[b] -> t_all [128,(8,128)] fp32
    PE    8 transposes (f32r, raw) -> ptT psum
    ACT   evacuate + downcast -> tT bf16 sbuf
    DVE   t2T = tT*tT (bf16 2x mode)
    PE    8 mini-matmuls (ones rhs, ap=1) -> per-token sumsq in psum
    ACT   sqrt -> tnrm; DVE reciprocal -> invt [128,8]
    PE    8 dots matmuls (bf16): pdots[k,(n m)] = tT.T @ iT_b
    DVE+POOL  dn = pdots * invt (split halves, evacuates psum, bf16)
    DVE   st_t = max over m (free reduce)
    POOL  partition-max over k-partitions -> nmax (replicated)
    DVE   max-tree over n (bf16 2x) -> st_i
    every 4 batches: ACT sqrt(2-2x) on staged maxes, DVE accumulate
Host combines the 8 per-core partial sums.
"""

import numpy as np
from contextlib import ExitStack

import concourse.bass as bass
import concourse.bass_isa as bass_isa
import concourse.mybir as mybir
import concourse.tile as tile
from concourse import bacc
from concourse.bass_utils import run_bass_kernel_spmd

N_CORES = 8
B, K, M, D = 512, 1024, 64, 128
B_LOC = B // N_CORES          # 64 batches per core
KT = K // 128                 # 8 token tiles of [128, D] per batch
ALPHA_T_TO_I = 0.3
STG = 4                       # sqrt-staging factor (batches per sqrt op)
LAG = 3                       # tail lags front by LAG batches

F32 = mybir.dt.float32
F32R = mybir.dt.float32r
BF16 = mybir.dt.bfloat16
AX = mybir.AxisListType
OP = mybir.AluOpType
ACT = mybir.ActivationFunctionType
RED = bass_isa.ReduceOp


def build(b_loc=B_LOC):
    assert b_loc % STG == 0 and b_loc % 16 == 0
    nc = bacc.Bacc(
        "TRN2",
        target_bir_lowering=False,
        debug=False,
        num_devices=N_CORES,
    )
    tokens = nc.dram_tensor("tokens", [b_loc, K, D], F32, kind="ExternalInput").ap()
    interests = nc.dram_tensor(
        "interests", [b_loc, M, D], F32, kind="ExternalInput"
    ).ap()
    out = nc.dram_tensor("out", [1, 2], F32, kind="ExternalOutput").ap()

    NG = b_loc * M // 128     # interest row-groups of 128 (= b_loc/2)
    NCHUNK = 4                # phase-0 chunks
    CG = NG // NCHUNK         # groups per chunk (8)

    with ExitStack() as ctx:
        tc = ctx.enter_context(tile.TileContext(nc))
        singles = ctx.enter_context(tc.tile_pool(name="singles", bufs=1))
        tok_pool = ctx.enter_context(tc.tile_pool(name="tok", bufs=5))
        tT_pool = ctx.enter_context(tc.tile_pool(name="tT", bufs=3))
        t2_pool = ctx.enter_context(tc.tile_pool(name="t2", bufs=2))
        dn_pool = ctx.enter_context(tc.tile_pool(name="dn", bufs=6))
        nm_pool = ctx.enter_context(tc.tile_pool(name="nm", bufs=3))
        tr_pool = ctx.enter_context(tc.tile_pool(name="tr", bufs=3))
        small = ctx.enter_context(tc.tile_pool(name="small", bufs=8))
        stage = ctx.enter_context(tc.tile_pool(name="stage", bufs=3))
        p_tT = ctx.enter_context(tc.tile_pool(name="p_tT", bufs=2, space="PSUM"))
        p_dots = ctx.enter_context(tc.tile_pool(name="p_dots", bufs=1, space="PSUM"))
        p_sum = ctx.enter_context(tc.tile_pool(name="p_sum", bufs=1, space="PSUM"))

        # identities for PE transposes (f32r for tokens, bf16 for interests)
        id_f32 = singles.tile([128, 128], F32)
        nc.gpsimd.memset(id_f32, 0.0)
        nc.gpsimd.affine_select(
            out=id_f32, in_=id_f32, compare_op=OP.not_equal, fill=1.0,
            base=0, pattern=[[-1, 128]], channel_multiplier=1,
        )
        id_bf = singles.tile([128, 128], BF16)
        nc.gpsimd.memset(id_bf, 0.0)
        nc.gpsimd.affine_select(
            out=id_bf, in_=id_bf, compare_op=OP.not_equal, fill=1.0,
            base=0, pattern=[[-1, 128]], channel_multiplier=1,
        )
        ones_bf = singles.tile([128, 1], BF16)
        nc.vector.memset(ones_bf, 1.0)
        ones_f = singles.tile([128, 1], F32)
        nc.vector.memset(ones_f, 1.0)
        two = singles.tile([128, 1], F32)
        nc.vector.memset(two, 2.0)
        acc_t = singles.tile([128, STG * KT], F32)
        acc_i = singles.tile([128, STG * M], F32)
        nc.vector.memset(acc_t, 0.0)
        nc.vector.memset(acc_i, 0.0)

        # ---------- phase 0: all interests -> normalized iT_all (bf16) ----------
        # chunked; chunks are issued interleaved with the first batch pairs
        i_flat = interests.rearrange("b m d -> (b m) d").rearrange(
            "(g p) d -> p g d", p=128
        )  # [128, NG, 128]
        i_all = singles.tile([128, NG, D], F32)
        iT_all = singles.tile([128, b_loc, M], BF16)   # [d, (b, m)]

        def phase0_chunk(c):
            sl = i_all[:, c * CG:(c + 1) * CG, :]          # [128, 8, 128]
            nc.sync.dma_start(out=sl, in_=i_flat[:, c * CG:(c + 1) * CG, :])
            i2 = t2_pool.tile([128, CG, D], BF16, tag="i2")
            nc.scalar.square(i2, sl)
            isum = small.tile([128, CG], F32, tag="isum")
            nc.vector.tensor_reduce(isum, i2, axis=AX.X, op=OP.add)
            inrm = small.tile([128, CG], F32, tag="inrm")
            nc.scalar.sqrt(inrm, isum)
            invi = small.tile([128, CG], F32, tag="invi")
            nc.vector.reciprocal(invi, inrm)
            i_nb = tT_pool.tile([128, CG, D], BF16, tag="i_nb")
            nc.gpsimd.tensor_mul(i_nb, sl, invi.broadcast_to([128, CG, D]))
            ptiT = p_sum.tile([128, CG, 128], BF16, tag="ptiT")
            for j in range(CG):
                nc.tensor.transpose(ptiT[:, j, :], i_nb[:, j, :], id_bf)
            dst = iT_all[:, c * 2 * CG:(c + 1) * 2 * CG, :].rearrange(
                "p a b -> p (a b)"
            )
            nc.scalar.copy(dst, ptiT.rearrange("p a b -> p (a b)"))

        # ---------- main loop (batch pairs, software-pipelined) ----------
        dn_of = {}
        st_of = {}
        NP = b_loc // 2

        def front_pair(pp):
            tT2 = tT_pool.tile([128, 2, KT, 128], BF16, tag="tT2")
            t2T2 = t2_pool.tile([128, 2, KT, 128], BF16, tag="t2T2")
            for pr in range(2):
                b = 2 * pp + pr
                t_all = tok_pool.tile([128, KT, D], F32)
                nc.sync.dma_start(
                    out=t_all, in_=tokens[b].rearrange("(n p) d -> p n d", p=128)
                )
                # transposes of raw token tiles (fp32)
                ptT = p_tT.tile([128, KT, 128], F32)
                for n in range(KT):
                    nc.tensor.transpose(ptT[:, n, :], t_all[:, n, :], id_f32)
                # evacuate + downcast on ACT
                nc.scalar.copy(
                    tT2[:, pr].rearrange("p a b -> p (a b)"),
                    ptT.rearrange("p a b -> p (a b)"),
                )
            # squares (split DVE bf16-2x / Pool) for the whole pair
            flat_t = tT2.rearrange("p r a b -> p (r a b)")
            flat_t2 = t2T2.rearrange("p r a b -> p (r a b)")
            nc.vector.tensor_mul(flat_t2[:, 0:1344], flat_t[:, 0:1344],
                                 flat_t[:, 0:1344])
            nc.gpsimd.tensor_mul(flat_t2[:, 1344:2048], flat_t[:, 1344:2048],
                                 flat_t[:, 1344:2048])
            # per-token sumsq via mini-matmuls (ap=1, nearly free on PE)
            ptsum = p_sum.tile([128, KT, 2], F32)
            for pr in range(2):
                for n in range(KT):
                    nc.tensor.matmul(
                        ptsum[:, n, pr:pr + 1], lhsT=t2T2[:, pr, n, :],
                        rhs=ones_bf, start=True, stop=True,
                    )
            tnrm = small.tile([128, KT * 2], F32, tag="tnrm")
            nc.scalar.sqrt(tnrm, ptsum.rearrange("p a b -> p (a b)"))
            invt = small.tile([128, KT * 2], F32, tag="invt")
            nc.vector.reciprocal(invt, tnrm)
            # dots (raw bf16 tokens x normalized interests)
            pdots = p_dots.tile([128, KT, 2, M], F32, tag="pd")
            for pr in range(2):
                iT = iT_all[:, 2 * pp + pr, :]
                for n in range(KT):
                    nc.tensor.matmul(
                        pdots[:, n, pr, :], lhsT=tT2[:, pr, n, :], rhs=iT,
                        start=True, stop=True,
                    )
            # normalize + evacuate psum (DVE; Pool cannot read PSUM)
            dn = dn_pool.tile([128, KT, 2, M], BF16)
            nc.vector.tensor_mul(
                dn, pdots,
                invt.rearrange("p (a b) -> p a b", a=KT).broadcast_to(
                    [128, KT, 2, M]),
            )
            dn_of[pp] = dn

        def tail_pair(pq):
            s2 = (2 * pq) % STG
            g = (2 * pq) // STG
            if s2 == 0:
                st_t_new = stage.tile([128, STG, KT], F32, tag="st_t")
                st_i_new = stage.tile([128, STG, M], BF16, tag="st_i")
                st_of[g] = (st_t_new, st_i_new)
            st_t, st_i = st_of[g]
            dn = dn_of.pop(pq)         # [128, KT, 2, M]
            # per-token max over m, both batches at once
            nc.vector.tensor_reduce(
                st_t[:, s2:s2 + 2, :].rearrange("p r n -> p n r"),
                dn, axis=AX.X, op=OP.max)
            # per-interest: max-tree level 1, partition-max on half, rest
            m1 = tr_pool.tile([128, KT // 2, 2, M], BF16, tag="m1")
            nc.vector.tensor_max(m1, dn[:, 0:4], dn[:, 4:8])
            nmax = nm_pool.tile([128, KT // 2, 2, M], BF16)
            nc.gpsimd.partition_all_reduce(
                nmax.rearrange("p a r b -> p (a r b)"),
                m1.rearrange("p a r b -> p (a r b)"),
                channels=128, reduce_op=RED.max,
            )
            m2 = tr_pool.tile([128, KT // 4, 2, M], BF16, tag="m2")
            nc.vector.tensor_max(m2, nmax[:, 0:2], nmax[:, 2:4])
            nc.vector.tensor_max(st_i[:, s2:s2 + 2, :], m2[:, 0], m2[:, 1])
            if s2 + 2 == STG:
                del st_of[g]
                dts = stage.tile([128, STG * KT], F32, tag="dts")
                nc.scalar.activation(
                    dts, st_t.rearrange("p a b -> p (a b)"),
                    ACT.Sqrt, bias=two[:], scale=-2.0,
                )
                nc.gpsimd.tensor_add(acc_t, acc_t, dts)
                dis = stage.tile([128, STG * M], F32, tag="dis")
                nc.scalar.activation(
                    dis, st_i.rearrange("p a b -> p (a b)"),
                    ACT.Sqrt, bias=two[:], scale=-2.0,
                )
                nc.gpsimd.tensor_add(acc_i, acc_i, dis)

        LAGP = 2
        CHUNK_AT = {0: 0, 3: 1, 6: 2, 9: 3}
        for vp in range(NP + LAGP):
            if vp in CHUNK_AT:
                phase0_chunk(CHUNK_AT[vp])
            if vp >= LAGP:
                tail_pair(vp - LAGP)
            if vp < NP:
                front_pair(vp)

        # ---------- final reductions ----------
        red_t = singles.tile([128, 1], F32)
        nc.vector.tensor_reduce(red_t, acc_t, axis=AX.X, op=OP.add)
        pfin3 = p_dots.tile([128, KT, 2, M], F32, tag="pd")
        pfin = pfin3.rearrange("p a b c -> p (a b c)")
        nc.tensor.matmul(pfin[:1, :1], lhsT=ones_f, rhs=red_t, start=True, stop=True)
        red_i = singles.tile([128, 1], F32)
        nc.vector.tensor_reduce(red_i, acc_i, axis=AX.X, op=OP.add)
        out_sb = small.tile([1, 2], F32, tag="out_sb")
        nc.scalar.copy(out_sb[:, 0:1], pfin[:1, :1])
        nc.scalar.copy(out_sb[:, 1:2], red_i[0:1, :])
        nc.sync.dma_start(out=out, in_=out_sb)

    nc.compile()
    return nc


_NC_CACHE = None


def _get_nc():
    global _NC_CACHE
    if _NC_CACHE is None:
        _NC_CACHE = build()
    return _NC_CACHE


def kernel(tokens: np.ndarray, interests: np.ndarray, _trace=False) -> np.ndarray:
    tokens = np.ascontiguousarray(tokens, dtype=np.float32)
    interests = np.ascontiguousarray(interests, dtype=np.float32)
    assert tokens.shape == (B, K, D) and interests.shape == (B, M, D)

    nc = _get_nc()
    in_maps = [
        {
            "tokens": tokens[c * B_LOC:(c + 1) * B_LOC],
            "interests": interests[c * B_LOC:(c + 1) * B_LOC],
        }
        for c in range(N_CORES)
    ]
    res = run_bass_kernel_spmd(
        nc, in_maps, core_ids=list(range(N_CORES)), trace=_trace
    )
    sum_t = 0.0  # sum over all (b, k) of min_m dist
    sum_i = 0.0  # sum over all (b, m) of min_k dist
    for r in res.results:
        sum_t += float(r["out"][0, 0])
        sum_i += float(r["out"][0, 1])
    loss = sum_i / (B * M) + ALPHA_T_TO_I * sum_t / (B * K)
    kernel.last_results = res
    return np.array(loss, dtype=np.float32)
